# revision 24
# baseline (speedup 1.0000x reference)
"""AttentiveTransformer v4.3: software-pipelined, balanced-engine kernel.

Per core: 8192 rows = 64 ghost-BN chunks of 128 rows, paired for stats.
Host centers feat per chunk (BN mean term vanishes), casts fp16; final
max(z - tau, 0) thresholding runs on host from device z + ntau.

Emission is software-pipelined: GEMM-phase(pair cp) is interleaved with
tail-phase(pair cp-1) in program order so the tile scheduler overlaps the
PE-heavy and post-stats phases.

GEMM-phase per chunk (row layout [128 rows, 2048 G], 4 subtiles of 512):
  PE : x = fcT.T @ WT (16 fp16 matmuls, fp32 PSUM)
       + 1 selector-matmul per subtile -> colsum(x^2) on vb8 [8,512] psum
  Act: xsq = Square(px) fp16; evict subtiles 0-2 (Copy -> x16 fp16)
  DVE: evict subtile 3; y = x16 * p16 (fp16 2x, early)
tail-phase per pair:
  Act: sqrt(vb8/128+eps); DVE: reciprocal; Act: cast f16
  DMA: gather s8h -> DRAM row; broadcast-read back as bc [128,2048] f16
  Q7 : z = y * bc (one tensor_tensor per chunk)
  DVE: m8 = top8(z); cumsum; ntau[:,c] = -max((cs-1)/k)
  z stored fp16 to DRAM pre-threshold (act queue).
"""
import numpy as np

import concourse.bass as bass
import concourse.bacc as bacc
import concourse.tile as tile
from concourse import mybir
from concourse.bass_utils import run_bass_kernel_spmd

F32 = mybir.dt.float32
F16 = mybir.dt.float16
BN_EPS = 1e-5

B_FULL, IN, G = 65536, 512, 2048
N_CORES = 8
P = 128
NT = 4          # 4 n-subtiles of 512 over G
KT = 4          # 4 k-tiles of 128 over IN
ACT = mybir.ActivationFunctionType
ALU = mybir.AluOpType


def build(n_tiles, gamma_trivial):
    from contextlib import ExitStack
    assert n_tiles % 2 == 0
    n_pairs = n_tiles // 2
    nc = bacc.Bacc()
    rows = n_tiles * P
    ft_d = nc.dram_tensor("ft", [IN, rows], F16, kind="ExternalInput")
    wt_d = nc.dram_tensor("wt", [IN, G], F16, kind="ExternalInput")
    p_d = nc.dram_tensor("p", [rows, G], F16, kind="ExternalInput")
    sel_d = nc.dram_tensor("sel64", [P, 64], F16, kind="ExternalInput")
    rinv_d = nc.dram_tensor("rinv8", [P, 16], F32, kind="ExternalInput")
    g8_d = nc.dram_tensor("g8", [8, 512], F32, kind="ExternalInput")
    z_d = nc.dram_tensor("z", [rows, G], F16, kind="ExternalOutput")
    ntau_d = nc.dram_tensor("ntau", [P, n_tiles], F32, kind="ExternalOutput")
    abs_d = nc.dram_tensor("abscratch", [n_pairs, 4096], F16, kind="Internal")

    with tile.TileContext(nc) as tc, ExitStack() as ctx:
        singles = ctx.enter_context(tc.tile_pool(name="singles", bufs=1))
        ppool = ctx.enter_context(tc.tile_pool(name="ppool", bufs=4))
        xsqpool = ctx.enter_context(tc.tile_pool(name="xsqpool", bufs=6))
        x16pool = ctx.enter_context(tc.tile_pool(name="x16pool", bufs=3))
        ypool = ctx.enter_context(tc.tile_pool(name="ypool", bufs=6))
        bcpool = ctx.enter_context(tc.tile_pool(name="bcpool", bufs=3))
        zpool = ctx.enter_context(tc.tile_pool(name="zpool", bufs=6))
        stpool = ctx.enter_context(tc.tile_pool(name="stpool", bufs=4))
        smpool = ctx.enter_context(tc.tile_pool(name="smpool", bufs=4))
        ps_x = ctx.enter_context(tc.tile_pool(name="ps_x", bufs=6, space="PSUM"))
        ps_v = ctx.enter_context(tc.tile_pool(name="ps_v", bufs=2, space="PSUM"))

        # ---- resident constants / weights / features ----
        ftk = []
        for k in range(KT):
            t = singles.tile([P, rows], F16, tag=f"ft{k}", name=f"ft{k}")
            nc.sync.dma_start(t[:], ft_d[k * P:(k + 1) * P, :])
            ftk.append(t)
        wtk = []
        for k in range(KT):
            t = singles.tile([P, G], F16, tag=f"wt{k}", name=f"wt{k}")
            nc.sync.dma_start(t[:], wt_d[k * P:(k + 1) * P, :])
            wtk.append(t)
        sel64 = singles.tile([P, 64], F16)
        nc.sync.dma_start(sel64[:], sel_d[:])
        rinv16 = singles.tile([P, 16], F32)
        nc.sync.dma_start(rinv16[:], rinv_d[:])
        zeros8 = singles.tile([P, 8], F16)
        nc.vector.memset(zeros8[:], 0.0)
        eps8 = singles.tile([8, 1], F32)
        nc.vector.memset(eps8[:], BN_EPS)
        ntau_all = singles.tile([P, n_tiles], F32)
        g8 = singles.tile([8, 512], F32)
        if not gamma_trivial:
            nc.sync.dma_start(g8[:], g8_d[:])

        state = {}          # pair cp -> dict with vb8, ys

        def gemm_phase(cp):
            tiles = (2 * cp, 2 * cp + 1)
            vb8 = ps_v.tile([8, 512], F32, tag="vb", name=f"vb{cp}")
            ys = {}
            for ti, c in enumerate(tiles):
                p16 = ppool.tile([P, G], F16, tag="p16", name=f"p16_{c}")
                nc.sync.dma_start(p16[:], p_d[c * P:(c + 1) * P, :])
                x16 = x16pool.tile([P, G], F16, tag="x16", name=f"x16_{c}")
                y = ypool.tile([P, G], F16, tag="y", name=f"y{c}")
                for n in range(NT):
                    s = 4 * ti + n
                    px = ps_x.tile([P, 512], F32, tag="px", name=f"px{s}")
                    for k in range(KT):
                        nc.tensor.matmul(px[:], ftk[k][:, c * P:(c + 1) * P],
                                         wtk[k][:, n * 512:(n + 1) * 512],
                                         start=(k == 0), stop=(k == KT - 1))
                    xsq = xsqpool.tile([P, 512], F16, tag="xsq",
                                       name=f"xsq{s}")
                    nc.scalar.activation(xsq[:], px[:], ACT.Square)
                    nc.tensor.matmul(vb8[:], sel64[:, 8 * s:8 * (s + 1)],
                                     xsq[:], start=(s == 0), stop=(s == 7))
                    # Act evicts every subtile: its queue has only short,
                    # wait-free ops, so PSUM banks recycle at PE pace
                    nc.scalar.activation(x16[:, n * 512:(n + 1) * 512],
                                         px[:], ACT.Copy)
                # y = x * priors in one fp16 2x pass
                nc.vector.tensor_tensor(y[:], x16[:], p16[:], op=ALU.mult)
                ys[ti] = y
            state[cp] = (vb8, ys)

        zstate = {}

        def scale_phase(cp):
            tiles = (2 * cp, 2 * cp + 1)
            vb8, ys = state.pop(cp)
            sig8 = stpool.tile([8, 512], F32, tag="sig", name=f"sig{cp}")
            nc.scalar.activation(sig8[:], vb8[:], ACT.Sqrt, bias=eps8[:],
                                 scale=1.0 / P)
            s8 = stpool.tile([8, 512], F32, tag="s8", name=f"s8_{cp}")
            nc.vector.reciprocal_approx_fast(s8[:], sig8[:])
            if not gamma_trivial:
                nc.vector.tensor_tensor(s8[:], s8[:], g8[:], op=ALU.mult)
            s8h = stpool.tile([8, 512], F16, tag="s8h", name=f"s8h{cp}")
            nc.vector.tensor_copy(s8h[:], s8[:])
            nc.gpsimd.dma_start(abs_d[cp:cp + 1, :], s8h[:])

            zs = {}
            for ti, c in enumerate(tiles):
                bc = bcpool.tile([P, G], F16, tag="bc", name=f"bc{c}")
                nc.gpsimd.dma_start(
                    bc[:],
                    abs_d[cp:cp + 1, ti * G:(ti + 1) * G].to_broadcast([P, G]))
                z16 = zpool.tile([P, G], F16, tag="z", name=f"z{c}")
                nc.gpsimd.tensor_tensor(z16[:], ys[ti][:], bc[:], op=ALU.mult)
                zs[ti] = z16
            zstate[cp] = zs

        def reduce_phase(cp):
            tiles = (2 * cp, 2 * cp + 1)
            zs = zstate.pop(cp)
            cs2 = smpool.tile([P, 16], F32, tag="cs2", name=f"cs2_{cp}")
            for ti, c in enumerate(tiles):
                z16 = zs[ti]
                # subtile-granular top8 then combine: shorter DVE ops avoid
                # head-of-line blocking of px-freeing y-multiplies
                m32 = smpool.tile([P, 32], F16, tag=f"m32_{ti}",
                                  name=f"m32_{c}")
                for n in range(NT):
                    nc.vector.max(m32[:, 8 * n:8 * (n + 1)],
                                  z16[:, n * 512:(n + 1) * 512])
                m8 = smpool.tile([P, 8], F16, tag=f"m8_{ti}", name=f"m8_{c}")
                nc.vector.max(m8[:], m32[:])
                nc.vector.tensor_tensor_scan(cs2[:, 8 * ti:8 * (ti + 1)],
                                             m8[:], zeros8[:], 0.0,
                                             op0=ALU.add, op1=ALU.bypass)
                # out-DMA issued from the gpsimd queue: producer (z-mult) is
                # the previous gpsimd op, so the issue never parks the queue
                nc.gpsimd.dma_start(z_d[c * P:(c + 1) * P, :], z16[:])
            taur2 = smpool.tile([P, 16], F32, tag="tr2", name=f"tr2_{cp}")
            nc.vector.scalar_tensor_tensor(taur2[:], in0=cs2[:], scalar=-1.0,
                                           in1=rinv16[:], op0=ALU.add,
                                           op1=ALU.mult)
            for ti, c in enumerate(tiles):
                nc.vector.tensor_reduce(ntau_all[:, c:c + 1],
                                        taur2[:, 8 * ti:8 * (ti + 1)],
                                        axis=mybir.AxisListType.X,
                                        op=ALU.max, negate=True)

        # ---- 3-stage software-pipelined emission; reduce is emitted BEFORE
        # scale so the DVE queue hits max8(cp-2) (long-ready z) before it
        # parks on anything fresh ----
        for cp in range(n_pairs + 2):
            if cp < n_pairs:
                gemm_phase(cp)
            if cp >= 2:
                reduce_phase(cp - 2)
            if 1 <= cp < n_pairs + 1:
                scale_phase(cp - 1)
        nc.sync.dma_start(ntau_d[:], ntau_all[:])

    nc.finalize()
    return nc


_CACHE = {}


def _host_prep(priors, feat, W, gamma):
    # center per ghost chunk in fp32, cast fp16
    fc = feat.reshape(-1, P, IN)
    fc = fc - fc.mean(axis=1, keepdims=True)
    fc16 = fc.astype(np.float16).reshape(-1, IN)
    fT = np.ascontiguousarray(fc16.T)                  # [IN, B] fp16
    wt = np.ascontiguousarray(W.T.astype(np.float16))  # [IN, G]
    p16 = priors.astype(np.float16)
    sel64 = np.zeros((P, 64), np.float16)
    for s in range(8):
        sel64[:, 8 * s + s] = 1.0
    rinv8 = np.broadcast_to(
        np.tile(1.0 / np.arange(1, 9, dtype=np.float32), 2), (P, 16)).copy()
    g8 = np.ascontiguousarray(
        np.tile(gamma.reshape(4, 512), (2, 1)).astype(np.float32))
    return fT, wt, p16, sel64, rinv8, g8


def kernel(priors, processed_feat, W, gamma, beta):
    priors = np.ascontiguousarray(priors, dtype=np.float32)
    feat = np.ascontiguousarray(processed_feat, dtype=np.float32)
    W = np.ascontiguousarray(W, dtype=np.float32)
    gamma = np.asarray(gamma, dtype=np.float32)
    beta = np.asarray(beta, dtype=np.float32)
    assert bool(np.all(beta == 0.0)), "beta != 0 path not implemented"
    gamma_trivial = bool(np.all(gamma == 1.0))

    B = feat.shape[0]
    shard = B // N_CORES
    n_tiles = shard // P

    key = (n_tiles, gamma_trivial)
    if key not in _CACHE:
        _CACHE[key] = build(*key)
    nc = _CACHE[key]

    fT, wt, p16, sel64, rinv8, g8 = _host_prep(priors, feat, W, gamma)
    in_maps = []
    for i in range(N_CORES):
        in_maps.append({
            "ft": np.ascontiguousarray(fT[:, i * shard:(i + 1) * shard]),
            "wt": wt,
            "p": p16[i * shard:(i + 1) * shard],
            "sel64": sel64,
            "rinv8": rinv8,
            "g8": g8,
        })
    res = run_bass_kernel_spmd(nc, in_maps, core_ids=list(range(N_CORES)))
    out = np.empty((B, G), np.float32)
    for i, r in enumerate(res.results):
        z = r["z"]                       # [shard, G] fp16
        ntau = r["ntau"]                 # [P, n_tiles] f32
        ntau_rows = ntau.T.reshape(-1)   # row within shard = c*128 + p
        o = out[i * shard:(i + 1) * shard]
        o[:] = z.astype(np.float32)
        o += ntau_rows[:, None]
        np.maximum(o, 0.0, out=o)
    return out


# revision 27
# speedup vs baseline: 1.0551x; 1.0551x over previous
"""AttentiveTransformer v4.3: software-pipelined, balanced-engine kernel.

Per core: 8192 rows = 64 ghost-BN chunks of 128 rows, paired for stats.
Host centers feat per chunk (BN mean term vanishes), casts fp16; final
max(z - tau, 0) thresholding runs on host from device z + ntau.

Emission is software-pipelined: GEMM-phase(pair cp) is interleaved with
tail-phase(pair cp-1) in program order so the tile scheduler overlaps the
PE-heavy and post-stats phases.

GEMM-phase per chunk (row layout [128 rows, 2048 G], 4 subtiles of 512):
  PE : x = fcT.T @ WT (16 fp16 matmuls, fp32 PSUM)
       + 1 selector-matmul per subtile -> colsum(x^2) on vb8 [8,512] psum
  Act: xsq = Square(px) fp16; evict subtiles 0-2 (Copy -> x16 fp16)
  DVE: evict subtile 3; y = x16 * p16 (fp16 2x, early)
tail-phase per pair:
  Act: sqrt(vb8/128+eps); DVE: reciprocal; Act: cast f16
  DMA: gather s8h -> DRAM row; broadcast-read back as bc [128,2048] f16
  Q7 : z = y * bc (one tensor_tensor per chunk)
  DVE: m8 = top8(z); cumsum; ntau[:,c] = -max((cs-1)/k)
  z stored fp16 to DRAM pre-threshold (act queue).
"""
import numpy as np

import concourse.bass as bass
import concourse.bacc as bacc
import concourse.tile as tile
from concourse import mybir
from concourse.bass_utils import run_bass_kernel_spmd

F32 = mybir.dt.float32
F16 = mybir.dt.float16
BN_EPS = 1e-5

B_FULL, IN, G = 65536, 512, 2048
N_CORES = 8
P = 128
NT = 4          # 4 n-subtiles of 512 over G
KT = 4          # 4 k-tiles of 128 over IN
ACT = mybir.ActivationFunctionType
ALU = mybir.AluOpType


def build(n_tiles, gamma_trivial):
    from contextlib import ExitStack
    assert n_tiles % 2 == 0
    n_pairs = n_tiles // 2
    nc = bacc.Bacc()
    rows = n_tiles * P
    ft_d = nc.dram_tensor("ft", [IN, rows], F16, kind="ExternalInput")
    wt_d = nc.dram_tensor("wt", [IN, G], F16, kind="ExternalInput")
    p_d = nc.dram_tensor("p", [rows, G], F16, kind="ExternalInput")
    sel_d = nc.dram_tensor("sel64", [P, 64], F16, kind="ExternalInput")
    rinv_d = nc.dram_tensor("rinv8", [P, 16], F32, kind="ExternalInput")
    g8_d = nc.dram_tensor("g8", [8, 512], F32, kind="ExternalInput")
    z_d = nc.dram_tensor("z", [rows, G], F16, kind="ExternalOutput")
    ntau_d = nc.dram_tensor("ntau", [P, n_tiles], F32, kind="ExternalOutput")
    abs_d = nc.dram_tensor("abscratch", [n_pairs, 4096], F16, kind="Internal")

    with tile.TileContext(nc) as tc, ExitStack() as ctx:
        singles = ctx.enter_context(tc.tile_pool(name="singles", bufs=1))
        ppool = ctx.enter_context(tc.tile_pool(name="ppool", bufs=4))
        xsqpool = ctx.enter_context(tc.tile_pool(name="xsqpool", bufs=8))
        x16pool = ctx.enter_context(tc.tile_pool(name="x16pool", bufs=3))
        ypool = ctx.enter_context(tc.tile_pool(name="ypool", bufs=8))
        bcpool = ctx.enter_context(tc.tile_pool(name="bcpool", bufs=3))
        zpool = ctx.enter_context(tc.tile_pool(name="zpool", bufs=6))
        stpool = ctx.enter_context(tc.tile_pool(name="stpool", bufs=4))
        smpool = ctx.enter_context(tc.tile_pool(name="smpool", bufs=4))
        ps_x = ctx.enter_context(tc.tile_pool(name="ps_x", bufs=6, space="PSUM"))
        ps_v = ctx.enter_context(tc.tile_pool(name="ps_v", bufs=2, space="PSUM"))

        # ---- resident constants / weights / features ----
        ftk = []
        for k in range(KT):
            t = singles.tile([P, rows], F16, tag=f"ft{k}", name=f"ft{k}")
            nc.sync.dma_start(t[:], ft_d[k * P:(k + 1) * P, :])
            ftk.append(t)
        wtk = []
        for k in range(KT):
            t = singles.tile([P, G], F16, tag=f"wt{k}", name=f"wt{k}")
            nc.sync.dma_start(t[:], wt_d[k * P:(k + 1) * P, :])
            wtk.append(t)
        sel64 = singles.tile([P, 64], F16)
        nc.sync.dma_start(sel64[:], sel_d[:])
        rinv16 = singles.tile([P, 16], F32)
        nc.sync.dma_start(rinv16[:], rinv_d[:])
        zeros8 = singles.tile([P, 8], F16)
        nc.vector.memset(zeros8[:], 0.0)
        eps8 = singles.tile([8, 1], F32)
        nc.vector.memset(eps8[:], BN_EPS)
        ntau_all = singles.tile([P, n_tiles], F32)
        g8 = singles.tile([8, 512], F32)
        if not gamma_trivial:
            nc.sync.dma_start(g8[:], g8_d[:])

        state = {}          # pair cp -> dict with vb8, ys

        def gemm_phase(cp):
            tiles = (2 * cp, 2 * cp + 1)
            vb8 = ps_v.tile([8, 512], F32, tag="vb", name=f"vb{cp}")
            ys = {}
            for ti, c in enumerate(tiles):
                p16 = ppool.tile([P, G], F16, tag="p16", name=f"p16_{c}")
                nc.sync.dma_start(p16[:], p_d[c * P:(c + 1) * P, :])
                x12 = x16pool.tile([P, 1024], F16, tag="x12", name=f"x12_{c}")
                y = ypool.tile([P, G], F16, tag="y", name=f"y{c}")
                for n in range(NT):
                    s = 4 * ti + n
                    px = ps_x.tile([P, 512], F32, tag="px", name=f"px{s}")
                    for k in range(KT):
                        nc.tensor.matmul(px[:], ftk[k][:, c * P:(c + 1) * P],
                                         wtk[k][:, n * 512:(n + 1) * 512],
                                         start=(k == 0), stop=(k == KT - 1))
                    xsq = xsqpool.tile([P, 512], F16, tag="xsq",
                                       name=f"xsq{s}")
                    nc.scalar.activation(xsq[:], px[:], ACT.Square)
                    nc.tensor.matmul(vb8[:], sel64[:, 8 * s:8 * (s + 1)],
                                     xsq[:], start=(s == 0), stop=(s == 7))
                    # y = x * priors, emitted per subtile so PSUM frees fast:
                    # subtiles 0-1 evict via Act (fp16 2x path), 2-3 straight
                    # from PSUM on DVE (PSUM reads skip SBUF ports)
                    if n < 2:
                        nc.scalar.activation(x12[:, n * 512:(n + 1) * 512],
                                             px[:], ACT.Copy)
                        if n == 1:
                            nc.vector.tensor_tensor(y[:, 0:1024], x12[:],
                                                    p16[:, 0:1024],
                                                    op=ALU.mult)
                    else:
                        nc.vector.tensor_tensor(y[:, n * 512:(n + 1) * 512],
                                                px[:],
                                                p16[:, n * 512:(n + 1) * 512],
                                                op=ALU.mult)
                ys[ti] = y
            state[cp] = (vb8, ys)

        zstate = {}

        def scale_phase(cp):
            tiles = (2 * cp, 2 * cp + 1)
            vb8, ys = state.pop(cp)
            sig8 = stpool.tile([8, 512], F32, tag="sig", name=f"sig{cp}")
            nc.scalar.activation(sig8[:], vb8[:], ACT.Sqrt, bias=eps8[:],
                                 scale=1.0 / P)
            s8 = stpool.tile([8, 512], F32, tag="s8", name=f"s8_{cp}")
            nc.vector.reciprocal_approx_fast(s8[:], sig8[:])
            if not gamma_trivial:
                nc.vector.tensor_tensor(s8[:], s8[:], g8[:], op=ALU.mult)
            s8h = stpool.tile([8, 512], F16, tag="s8h", name=f"s8h{cp}")
            nc.scalar.activation(s8h[:], s8[:], ACT.Copy)
            nc.scalar.dma_start(abs_d[cp:cp + 1, :], s8h[:])

            zs = {}
            for ti, c in enumerate(tiles):
                bc = bcpool.tile([P, G], F16, tag="bc", name=f"bc{c}")
                nc.gpsimd.dma_start(
                    bc[:],
                    abs_d[cp:cp + 1, ti * G:(ti + 1) * G].to_broadcast([P, G]))
                z16 = zpool.tile([P, G], F16, tag="z", name=f"z{c}")
                nc.gpsimd.tensor_tensor(z16[:], ys[ti][:], bc[:], op=ALU.mult)
                zs[ti] = z16
            zstate[cp] = zs

        def reduce_phase(cp):
            tiles = (2 * cp, 2 * cp + 1)
            zs = zstate.pop(cp)
            cs2 = smpool.tile([P, 16], F32, tag="cs2", name=f"cs2_{cp}")
            for ti, c in enumerate(tiles):
                z16 = zs[ti]
                # subtile-granular top8 then combine: shorter DVE ops avoid
                # head-of-line blocking of px-freeing y-multiplies
                m32 = smpool.tile([P, 32], F16, tag=f"m32_{ti}",
                                  name=f"m32_{c}")
                for n in range(NT):
                    nc.vector.max(m32[:, 8 * n:8 * (n + 1)],
                                  z16[:, n * 512:(n + 1) * 512])
                m8 = smpool.tile([P, 8], F16, tag=f"m8_{ti}", name=f"m8_{c}")
                nc.vector.max(m8[:], m32[:])
                nc.vector.tensor_tensor_scan(cs2[:, 8 * ti:8 * (ti + 1)],
                                             m8[:], zeros8[:], 0.0,
                                             op0=ALU.add, op1=ALU.bypass)
                # out-DMA issued from the gpsimd queue: producer (z-mult) is
                # the previous gpsimd op, so the issue never parks the queue
                nc.gpsimd.dma_start(z_d[c * P:(c + 1) * P, :], z16[:])
            taur2 = smpool.tile([P, 16], F32, tag="tr2", name=f"tr2_{cp}")
            nc.vector.scalar_tensor_tensor(taur2[:], in0=cs2[:], scalar=-1.0,
                                           in1=rinv16[:], op0=ALU.add,
                                           op1=ALU.mult)
            for ti, c in enumerate(tiles):
                nc.vector.tensor_reduce(ntau_all[:, c:c + 1],
                                        taur2[:, 8 * ti:8 * (ti + 1)],
                                        axis=mybir.AxisListType.X,
                                        op=ALU.max, negate=True)

        # ---- 3-stage software-pipelined emission; reduce is emitted BEFORE
        # scale so the DVE queue hits max8(cp-2) (long-ready z) before it
        # parks on anything fresh ----
        for cp in range(n_pairs + 2):
            if cp < n_pairs:
                gemm_phase(cp)
            if cp >= 2:
                reduce_phase(cp - 2)
            if 1 <= cp < n_pairs + 1:
                scale_phase(cp - 1)
        nc.sync.dma_start(ntau_d[:], ntau_all[:])

    nc.finalize()
    return nc


_CACHE = {}


def _host_prep(priors, feat, W, gamma):
    # center per ghost chunk in fp32, cast fp16
    fc = feat.reshape(-1, P, IN)
    fc = fc - fc.mean(axis=1, keepdims=True)
    fc16 = fc.astype(np.float16).reshape(-1, IN)
    fT = np.ascontiguousarray(fc16.T)                  # [IN, B] fp16
    wt = np.ascontiguousarray(W.T.astype(np.float16))  # [IN, G]
    p16 = priors.astype(np.float16)
    sel64 = np.zeros((P, 64), np.float16)
    for s in range(8):
        sel64[:, 8 * s + s] = 1.0
    rinv8 = np.broadcast_to(
        np.tile(1.0 / np.arange(1, 9, dtype=np.float32), 2), (P, 16)).copy()
    g8 = np.ascontiguousarray(
        np.tile(gamma.reshape(4, 512), (2, 1)).astype(np.float32))
    return fT, wt, p16, sel64, rinv8, g8


def kernel(priors, processed_feat, W, gamma, beta):
    priors = np.ascontiguousarray(priors, dtype=np.float32)
    feat = np.ascontiguousarray(processed_feat, dtype=np.float32)
    W = np.ascontiguousarray(W, dtype=np.float32)
    gamma = np.asarray(gamma, dtype=np.float32)
    beta = np.asarray(beta, dtype=np.float32)
    assert bool(np.all(beta == 0.0)), "beta != 0 path not implemented"
    gamma_trivial = bool(np.all(gamma == 1.0))

    B = feat.shape[0]
    shard = B // N_CORES
    n_tiles = shard // P

    key = (n_tiles, gamma_trivial)
    if key not in _CACHE:
        _CACHE[key] = build(*key)
    nc = _CACHE[key]

    fT, wt, p16, sel64, rinv8, g8 = _host_prep(priors, feat, W, gamma)
    in_maps = []
    for i in range(N_CORES):
        in_maps.append({
            "ft": np.ascontiguousarray(fT[:, i * shard:(i + 1) * shard]),
            "wt": wt,
            "p": p16[i * shard:(i + 1) * shard],
            "sel64": sel64,
            "rinv8": rinv8,
            "g8": g8,
        })
    res = run_bass_kernel_spmd(nc, in_maps, core_ids=list(range(N_CORES)))
    out = np.empty((B, G), np.float32)
    for i, r in enumerate(res.results):
        z = r["z"]                       # [shard, G] fp16
        ntau = r["ntau"]                 # [P, n_tiles] f32
        ntau_rows = ntau.T.reshape(-1)   # row within shard = c*128 + p
        o = out[i * shard:(i + 1) * shard]
        o[:] = z.astype(np.float32)
        o += ntau_rows[:, None]
        np.maximum(o, 0.0, out=o)
    return out


# revision 32
# speedup vs baseline: 1.0727x; 1.0167x over previous
"""AttentiveTransformer v4.3: software-pipelined, balanced-engine kernel.

Per core: 8192 rows = 64 ghost-BN chunks of 128 rows, paired for stats.
Host centers feat per chunk (BN mean term vanishes), casts fp16; final
max(z - tau, 0) thresholding runs on host from device z + ntau.

Emission is software-pipelined: GEMM-phase(pair cp) is interleaved with
tail-phase(pair cp-1) in program order so the tile scheduler overlaps the
PE-heavy and post-stats phases.

GEMM-phase per chunk (row layout [128 rows, 2048 G], 4 subtiles of 512):
  PE : x = fcT.T @ WT (16 fp16 matmuls, fp32 PSUM)
       + 1 selector-matmul per subtile -> colsum(x^2) on vb8 [8,512] psum
  Act: xsq = Square(px) fp16; evict subtiles 0-2 (Copy -> x16 fp16)
  DVE: evict subtile 3; y = x16 * p16 (fp16 2x, early)
tail-phase per pair:
  Act: sqrt(vb8/128+eps); DVE: reciprocal; Act: cast f16
  DMA: gather s8h -> DRAM row; broadcast-read back as bc [128,2048] f16
  Q7 : z = y * bc (one tensor_tensor per chunk)
  DVE: m8 = top8(z); cumsum; ntau[:,c] = -max((cs-1)/k)
  z stored fp16 to DRAM pre-threshold (act queue).
"""
import numpy as np

import concourse.bass as bass
import concourse.bacc as bacc
import concourse.tile as tile
from concourse import mybir
from concourse.bass_utils import run_bass_kernel_spmd

F32 = mybir.dt.float32
F16 = mybir.dt.float16
BN_EPS = 1e-5

B_FULL, IN, G = 65536, 512, 2048
N_CORES = 8
P = 128
NT = 4          # 4 n-subtiles of 512 over G
KT = 4          # 4 k-tiles of 128 over IN
ACT = mybir.ActivationFunctionType
ALU = mybir.AluOpType


def build(n_tiles, gamma_trivial):
    from contextlib import ExitStack
    assert n_tiles % 2 == 0
    n_pairs = n_tiles // 2
    nc = bacc.Bacc()
    rows = n_tiles * P
    ft_d = nc.dram_tensor("ft", [IN, rows], F16, kind="ExternalInput")
    wt_d = nc.dram_tensor("wt", [IN, G], F16, kind="ExternalInput")
    p_d = nc.dram_tensor("p", [rows, G], F16, kind="ExternalInput")
    sel_d = nc.dram_tensor("sel64", [P, 64], F16, kind="ExternalInput")
    rinv_d = nc.dram_tensor("rinv8", [P, 16], F32, kind="ExternalInput")
    g8_d = nc.dram_tensor("g8", [8, 512], F32, kind="ExternalInput")
    z_d = nc.dram_tensor("z", [rows, G], F16, kind="ExternalOutput")
    ntau_d = nc.dram_tensor("ntau", [P, n_tiles], F32, kind="ExternalOutput")
    abs_d = nc.dram_tensor("abscratch", [n_pairs, 4096], F16, kind="Internal")

    with tile.TileContext(nc) as tc, ExitStack() as ctx:
        singles = ctx.enter_context(tc.tile_pool(name="singles", bufs=1))
        ppool = ctx.enter_context(tc.tile_pool(name="ppool", bufs=4))
        xsqpool = ctx.enter_context(tc.tile_pool(name="xsqpool", bufs=8))
        x16pool = ctx.enter_context(tc.tile_pool(name="x16pool", bufs=3))
        ypool = ctx.enter_context(tc.tile_pool(name="ypool", bufs=8))
        bcpool = ctx.enter_context(tc.tile_pool(name="bcpool", bufs=3))
        zpool = ctx.enter_context(tc.tile_pool(name="zpool", bufs=6))
        stpool = ctx.enter_context(tc.tile_pool(name="stpool", bufs=4))
        smpool = ctx.enter_context(tc.tile_pool(name="smpool", bufs=4))
        ps_x = ctx.enter_context(tc.tile_pool(name="ps_x", bufs=6, space="PSUM"))
        ps_v = ctx.enter_context(tc.tile_pool(name="ps_v", bufs=2, space="PSUM"))

        # ---- resident constants / weights / features ----
        ftk = []
        for k in range(KT):
            t = singles.tile([P, rows], F16, tag=f"ft{k}", name=f"ft{k}")
            nc.sync.dma_start(t[:], ft_d[k * P:(k + 1) * P, :])
            ftk.append(t)
        wtk = []
        for k in range(KT):
            t = singles.tile([P, G], F16, tag=f"wt{k}", name=f"wt{k}")
            nc.sync.dma_start(t[:], wt_d[k * P:(k + 1) * P, :])
            wtk.append(t)
        sel64 = singles.tile([P, 64], F16)
        nc.sync.dma_start(sel64[:], sel_d[:])
        rinv16 = singles.tile([P, 16], F32)
        nc.sync.dma_start(rinv16[:], rinv_d[:])
        zeros8 = singles.tile([P, 8], F16)
        nc.vector.memset(zeros8[:], 0.0)
        eps8 = singles.tile([8, 1], F32)
        nc.vector.memset(eps8[:], BN_EPS)
        ntau_all = singles.tile([P, n_tiles], F32)
        g8 = singles.tile([8, 512], F32)
        if not gamma_trivial:
            nc.sync.dma_start(g8[:], g8_d[:])

        state = {}          # pair cp -> dict with vb8, ys

        def gemm_phase(cp):
            tiles = (2 * cp, 2 * cp + 1)
            vb8 = ps_v.tile([8, 512], F32, tag="vb", name=f"vb{cp}")
            ys = {}
            pending = []    # delayed selector matmuls: (s, xsq)

            def flush_sel():
                for s_p, xsq_p in pending:
                    nc.tensor.matmul(vb8[:], sel64[:, 8 * s_p:8 * (s_p + 1)],
                                     xsq_p[:], start=(s_p == 0),
                                     stop=(s_p == 7))
                pending.clear()
            for ti, c in enumerate(tiles):
                p16 = ppool.tile([P, G], F16, tag="p16", name=f"p16_{c}")
                nc.sync.dma_start(p16[:], p_d[c * P:(c + 1) * P, :])
                x12 = x16pool.tile([P, 1024], F16, tag="x12", name=f"x12_{c}")
                y = ypool.tile([P, G], F16, tag="y", name=f"y{c}")
                for n in range(NT):
                    s = 4 * ti + n
                    px = ps_x.tile([P, 512], F32, tag="px", name=f"px{s}")
                    for k in range(KT):
                        nc.tensor.matmul(px[:], ftk[k][:, c * P:(c + 1) * P],
                                         wtk[k][:, n * 512:(n + 1) * 512],
                                         start=(k == 0), stop=(k == KT - 1))
                    # selector matmul for the PREVIOUS subtile: one mm-group
                    # of slack so PE never waits on Act's Square
                    flush_sel()
                    xsq = xsqpool.tile([P, 512], F16, tag="xsq",
                                       name=f"xsq{s}")
                    nc.scalar.activation(xsq[:], px[:], ACT.Square)
                    pending.append((s, xsq))
                    # y = x * priors, emitted per subtile so PSUM frees fast:
                    # subtiles 0-1 evict via Act (fp16 2x path), 2-3 straight
                    # from PSUM on DVE (PSUM reads skip SBUF ports)
                    if n < 2:
                        nc.scalar.activation(x12[:, n * 512:(n + 1) * 512],
                                             px[:], ACT.Copy)
                        if n == 1:
                            nc.vector.tensor_tensor(y[:, 0:1024], x12[:],
                                                    p16[:, 0:1024],
                                                    op=ALU.mult)
                    else:
                        nc.vector.tensor_tensor(y[:, n * 512:(n + 1) * 512],
                                                px[:],
                                                p16[:, n * 512:(n + 1) * 512],
                                                op=ALU.mult)
                ys[ti] = y
            flush_sel()
            state[cp] = (vb8, ys)

        zstate = {}

        def scale_phase(cp):
            tiles = (2 * cp, 2 * cp + 1)
            vb8, ys = state.pop(cp)
            sig8 = stpool.tile([8, 512], F32, tag="sig", name=f"sig{cp}")
            nc.scalar.activation(sig8[:], vb8[:], ACT.Sqrt, bias=eps8[:],
                                 scale=1.0 / P)
            s8 = stpool.tile([8, 512], F32, tag="s8", name=f"s8_{cp}")
            nc.vector.reciprocal_approx_fast(s8[:], sig8[:])
            if not gamma_trivial:
                nc.vector.tensor_tensor(s8[:], s8[:], g8[:], op=ALU.mult)
            s8h = stpool.tile([8, 512], F16, tag="s8h", name=f"s8h{cp}")
            nc.scalar.activation(s8h[:], s8[:], ACT.Copy)
            nc.scalar.dma_start(abs_d[cp:cp + 1, :], s8h[:])

            zs = {}
            for ti, c in enumerate(tiles):
                bc = bcpool.tile([P, G], F16, tag="bc", name=f"bc{c}")
                nc.gpsimd.dma_start(
                    bc[:],
                    abs_d[cp:cp + 1, ti * G:(ti + 1) * G].to_broadcast([P, G]))
                z16 = zpool.tile([P, G], F16, tag="z", name=f"z{c}")
                nc.gpsimd.tensor_tensor(z16[:], ys[ti][:], bc[:], op=ALU.mult)
                zs[ti] = z16
            zstate[cp] = zs

        def reduce_phase(cp):
            tiles = (2 * cp, 2 * cp + 1)
            zs = zstate.pop(cp)
            cs2 = smpool.tile([P, 16], F32, tag="cs2", name=f"cs2_{cp}")
            for ti, c in enumerate(tiles):
                z16 = zs[ti]
                # subtile-granular top8 then combine: shorter DVE ops avoid
                # head-of-line blocking of px-freeing y-multiplies
                m32 = smpool.tile([P, 32], F16, tag=f"m32_{ti}",
                                  name=f"m32_{c}")
                for n in range(NT):
                    nc.vector.max(m32[:, 8 * n:8 * (n + 1)],
                                  z16[:, n * 512:(n + 1) * 512])
                m8 = smpool.tile([P, 8], F16, tag=f"m8_{ti}", name=f"m8_{c}")
                nc.vector.max(m8[:], m32[:])
                nc.vector.tensor_tensor_scan(cs2[:, 8 * ti:8 * (ti + 1)],
                                             m8[:], zeros8[:], 0.0,
                                             op0=ALU.add, op1=ALU.bypass)
                # out-DMA issued from the gpsimd queue: producer (z-mult) is
                # the previous gpsimd op, so the issue never parks the queue
                nc.gpsimd.dma_start(z_d[c * P:(c + 1) * P, :], z16[:])
            taur2 = smpool.tile([P, 16], F32, tag="tr2", name=f"tr2_{cp}")
            nc.vector.scalar_tensor_tensor(taur2[:], in0=cs2[:], scalar=-1.0,
                                           in1=rinv16[:], op0=ALU.add,
                                           op1=ALU.mult)
            for ti, c in enumerate(tiles):
                nc.vector.tensor_reduce(ntau_all[:, c:c + 1],
                                        taur2[:, 8 * ti:8 * (ti + 1)],
                                        axis=mybir.AxisListType.X,
                                        op=ALU.max, negate=True)

        # ---- 3-stage software-pipelined emission; reduce is emitted BEFORE
        # scale so the DVE queue hits max8(cp-2) (long-ready z) before it
        # parks on anything fresh ----
        for cp in range(n_pairs + 2):
            if cp < n_pairs:
                gemm_phase(cp)
            if cp >= 2:
                reduce_phase(cp - 2)
            if 1 <= cp < n_pairs + 1:
                scale_phase(cp - 1)
        nc.sync.dma_start(ntau_d[:], ntau_all[:])

    nc.finalize()
    return nc


_CACHE = {}


def _host_prep(priors, feat, W, gamma):
    # center per ghost chunk in fp32, cast fp16
    fc = feat.reshape(-1, P, IN)
    fc = fc - fc.mean(axis=1, keepdims=True)
    fc16 = fc.astype(np.float16).reshape(-1, IN)
    fT = np.ascontiguousarray(fc16.T)                  # [IN, B] fp16
    wt = np.ascontiguousarray(W.T.astype(np.float16))  # [IN, G]
    p16 = priors.astype(np.float16)
    sel64 = np.zeros((P, 64), np.float16)
    for s in range(8):
        sel64[:, 8 * s + s] = 1.0
    rinv8 = np.broadcast_to(
        np.tile(1.0 / np.arange(1, 9, dtype=np.float32), 2), (P, 16)).copy()
    g8 = np.ascontiguousarray(
        np.tile(gamma.reshape(4, 512), (2, 1)).astype(np.float32))
    return fT, wt, p16, sel64, rinv8, g8


def kernel(priors, processed_feat, W, gamma, beta):
    priors = np.ascontiguousarray(priors, dtype=np.float32)
    feat = np.ascontiguousarray(processed_feat, dtype=np.float32)
    W = np.ascontiguousarray(W, dtype=np.float32)
    gamma = np.asarray(gamma, dtype=np.float32)
    beta = np.asarray(beta, dtype=np.float32)
    assert bool(np.all(beta == 0.0)), "beta != 0 path not implemented"
    gamma_trivial = bool(np.all(gamma == 1.0))

    B = feat.shape[0]
    shard = B // N_CORES
    n_tiles = shard // P

    key = (n_tiles, gamma_trivial)
    if key not in _CACHE:
        _CACHE[key] = build(*key)
    nc = _CACHE[key]

    fT, wt, p16, sel64, rinv8, g8 = _host_prep(priors, feat, W, gamma)
    in_maps = []
    for i in range(N_CORES):
        in_maps.append({
            "ft": np.ascontiguousarray(fT[:, i * shard:(i + 1) * shard]),
            "wt": wt,
            "p": p16[i * shard:(i + 1) * shard],
            "sel64": sel64,
            "rinv8": rinv8,
            "g8": g8,
        })
    res = run_bass_kernel_spmd(nc, in_maps, core_ids=list(range(N_CORES)))
    out = np.empty((B, G), np.float32)
    for i, r in enumerate(res.results):
        z = r["z"]                       # [shard, G] fp16
        ntau = r["ntau"]                 # [P, n_tiles] f32
        ntau_rows = ntau.T.reshape(-1)   # row within shard = c*128 + p
        o = out[i * shard:(i + 1) * shard]
        o[:] = z.astype(np.float32)
        o += ntau_rows[:, None]
        np.maximum(o, 0.0, out=o)
    return out
